# revision 8
# baseline (speedup 1.0000x reference)
"""Trainium2 Bass kernel for nn_DFlashAttention_43774306681111.

Full-attention transformer block: QKV projection + per-head RMSNorm + neox
RoPE + GQA softmax attention (non-causal) + output projection.

Sharding (8 cores): 2-way data parallel over batch x 4-way tensor parallel
over heads. Core c handles batch c//4 and head group g=c%4 (q heads
4g..4g+3, kv head g). Each core computes a partial output [S, HID]
(its heads' contribution through Wo); the host sums the 4 partials per
batch. No device collectives.

Device layout: activations kept transposed ([dim, token], dim on
partitions) so every matmul contracts on the partition axis:
  Q^T = Wq_tile^T @ X^T          (stationary Wq tile, moving X^T tile)
  S^T[k,q] = K^T_tile^T @ Q^T    (contraction d=128, one matmul per tile)
  softmax over k (= partitions): one exp per KEY-TILE PAIR ([128,1024]
    ACT instruction over a 2-bank psum tile, amortizing the fixed
    activation overhead); the denominator is accumulated with bf16 DVE
    adds over exp pairs and reduced with two ones-vector matmuls per
    block (instead of one per key tile).
  ctx^T[d,q] = V_tile^T @ expS^T (V stationary [k_tok, d])
  out[tok,hid] = ctxT_tile^T @ Wo
V is transposed [d,tok]->[tok,d] by the DMA xbar (dma_start_transpose,
one per token block) -- no PE transposes, no psum evacuations for it.

RoPE: the head dims are PERMUTED host-side (Wq/Wk columns, cos/sin
tables, norm weights) so the rotation pair (i, i+64) sits on adjacent
partitions (2i, 2i+1). The half-swap is then a single DVE stream_shuffle
(even<->odd within each 32-partition quadrant) instead of two SBUF->SBUF
DMAs. Scores/outputs are unchanged because QK contracts over the (same)
permutation of both q and k, and v/ctx/Wo are untouched.

PSUM budget (8 banks): three 2-bank pair tiles (pA: phase-A acc q0/q1
then QK even pairs; pC: acc q2/q3 then QK odd pairs; pE: acc k/v then
the double-buffered ctx accumulator) + pG (softmax sums) + pH (rmsnorm
ssq + Wo output tiles).

Engine discipline: ACT runs ONLY exp in the attention phase (plus the
batched tb3 q-tail sqrts at one point and the Wo psum copies, which
share Exp's table set) so its table never thrashes. All other PSUM
evacuations go to DVE. Wo matmuls are spread one output tile (4
matmuls) per slot through the next q-block's PE stream so exp never
starves behind a long Wo burst.
"""
import numpy as np
from contextlib import ExitStack

import concourse.bass as bass
import concourse.tile as tile
from concourse import bacc, mybir
from concourse.bass_utils import run_bass_kernel_spmd

B, S, HID = 2, 2048, 2048
NH, NKV, D = 16, 4, 128
EPS = 1e-6
THETA = 1000000.0
SCALE = D ** -0.5

TP = 4                 # tensor-parallel groups (heads)
DP = 2                 # data-parallel over batch
HG = NH // TP          # q heads per core = 4
DQ = HG * D            # 512 q-proj cols per core
HALF = D // 2          # 64

F32 = mybir.dt.float32
F32R = mybir.dt.float32r
BF16 = mybir.dt.bfloat16
FP16 = mybir.dt.float16

MM_DT = F32R           # ctx / Wo matmul operand dtype
MM_NP = np.float32
PROJ_DT = FP16         # projection operand dtype (halves phase A DMA)
PROJ_NP = np.float16

HT = HID // 128        # 16 hid tiles
TBS = 512              # token block size
NTB = S // TBS         # 4 token blocks
KT = S // 128          # 16 key tiles
QB = S // TBS          # 4 query blocks
NDT = HG + 2           # 6 projection outputs: q0..q3, k, v^T

STAGGER = 5            # AV matmul lag behind QK/exp

SWAP_MASK = [i ^ 1 for i in range(32)]   # even<->odd partition swap

_cache = {}


def _build(skip_w=False):
    nc = bacc.Bacc(None, target_bir_lowering=False, debug=False)

    xt = nc.dram_tensor("xt", [HID, S], PROJ_DT, kind="ExternalInput")
    wq = nc.dram_tensor("wq", [HID, DQ], PROJ_DT, kind="ExternalInput")
    wk = nc.dram_tensor("wk", [HID, D], PROJ_DT, kind="ExternalInput")
    wv = nc.dram_tensor("wv", [HID, D], PROJ_DT, kind="ExternalInput")
    wo = nc.dram_tensor("wo", [DQ, HID], MM_DT, kind="ExternalInput")
    cos2 = nc.dram_tensor("cos2", [D, S], BF16, kind="ExternalInput")
    sin2 = nc.dram_tensor("sin2", [D, S], BF16, kind="ExternalInput")
    qnw = nc.dram_tensor("qnw", [D, 1], F32, kind="ExternalInput")
    knw = nc.dram_tensor("knw", [D, 1], F32, kind="ExternalInput")
    onesb_d = nc.dram_tensor("onesb", [128, 1], BF16, kind="ExternalInput")
    out = nc.dram_tensor("out", [S, HID], F32, kind="ExternalOutput")

    with tile.TileContext(nc) as tc, ExitStack() as ctx:
        const = ctx.enter_context(tc.tile_pool(name="const", bufs=1))
        big = ctx.enter_context(tc.tile_pool(name="big", bufs=1))
        blk = ctx.enter_context(tc.tile_pool(name="blk", bufs=8))
        outp = ctx.enter_context(tc.tile_pool(name="outp", bufs=3))
        scratch = ctx.enter_context(tc.tile_pool(name="scratch", bufs=2))
        rows = ctx.enter_context(tc.tile_pool(name="rows", bufs=2))
        psum = ctx.enter_context(tc.tile_pool(name="psum", bufs=1, space="PSUM"))

        # ---- constants ----
        onesb_col = const.tile([128, 1], BF16)
        nc.scalar.dma_start(out=onesb_col[:], in_=onesb_d[:])
        eps1 = const.tile([1, 1], F32)
        nc.vector.memset(eps1, EPS)
        qnw_sb = const.tile([D, 1], F32)
        nc.scalar.dma_start(out=qnw_sb[:], in_=qnw[:])
        knw_sb = const.tile([D, 1], F32)
        nc.scalar.dma_start(out=knw_sb[:], in_=knw[:])

        # ---- resident weights / big activations (tag-shared slots) ----
        wq_sb = big.tile([128, HT, DQ], PROJ_DT, tag="bigw")
        wk_sb = big.tile([128, HT, D], PROJ_DT, tag="wk")
        wv_sb = big.tile([128, HT, D], PROJ_DT, tag="wv")
        cos_sb = big.tile([D, S], BF16, tag="cosb")
        sin_sb = big.tile([D, S], BF16, tag="sinb")

        qT = big.tile([D, HG, S], BF16, tag="qT")        # Q^T per head
        kT = big.tile([D, S], BF16, tag="kT")            # K^T
        vT = big.tile([D, S], BF16, tag="vT")            # V^T (pre-transpose)
        v_sb = big.tile([128, KT, D], BF16, tag="v")     # V [tok, d] tiles

        def stationary(ht, dt):
            if dt < HG:
                return wq_sb[:, ht, dt * D:(dt + 1) * D]
            if dt == HG:
                return wk_sb[:, ht, :]
            return wv_sb[:, ht, :]

        # Deferred PE work from the rmsnorm tails: one ssq matmul + sqrt +
        # recip + broadcast + rope chain per projection output. Flushed at
        # spread-out points of the later PE stream.
        pending_pe = []

        def flush_pe(k=1):
            for _ in range(min(k, len(pending_pe))):
                pending_pe.pop(0)()

        # ---- phase A: projections ----
        for tb in range(NTB):
            tsl = slice(tb * TBS, (tb + 1) * TBS)
            pairs = [psum.tile([128, 2, TBS], F32, tag=t,
                               name=f"accp_{tb}_{t}")
                     for t in ("pA", "pC", "pE")]
            accs = [pairs[dt // 2][:, dt % 2, :] for dt in range(NDT)]
            for ht in range(HT):
                if tb == 0:
                    hsl = slice(ht * 128, (ht + 1) * 128)
                    nc.scalar.dma_start(out=wq_sb[:, ht, :], in_=wq[hsl, :])
                    nc.scalar.dma_start(out=wk_sb[:, ht, :], in_=wk[hsl, :])
                    nc.scalar.dma_start(out=wv_sb[:, ht, :], in_=wv[hsl, :])
                if ht == 1:
                    nc.scalar.dma_start(out=cos_sb[:, tsl], in_=cos2[:, tsl])
                    nc.scalar.dma_start(out=sin_sb[:, tsl], in_=sin2[:, tsl])
                if ht in (4, 6, 8, 10, 12):
                    flush_pe(1)  # previous tb's rmsnorm/rope tails
                xt_t = blk.tile([128, TBS], PROJ_DT, tag="xt", bufs=12,
                                name=f"xt_{tb}_{ht}")
                nc.sync.dma_start(out=xt_t[:], in_=xt[ht * 128:(ht + 1) * 128, tsl])
                for dt in range(NDT):
                    nc.tensor.matmul(accs[dt], stationary(ht, dt), xt_t[:],
                                     start=(ht == 0), stop=(ht == HT - 1))

            # ---- end of tb: evacuate psums fast (k first: its tail ssq
            # flushes earliest), then queue tail work ----
            raws = {}
            q2s = {}
            for i, dt in enumerate([HG, 0, 1, NDT - 1, 2, 3]):
                acc = accs[dt]
                if dt == NDT - 1:
                    # v^T: evac, then DMA-transpose this tb's chunk into
                    # v_sb [tok, d] on an idle DMA queue (no PE transposes)
                    nc.vector.tensor_copy(vT[:, tsl], acc)
                    nc.sync.dma_start_transpose(
                        v_sb[:, tb * (TBS // 128):(tb + 1) * (TBS // 128), :],
                        vT[:, tsl])
                    continue
                raw = scratch.tile([128, TBS], BF16, tag="raw", bufs=10,
                                   name=f"raw_{tb}_{dt}")
                nc.vector.tensor_copy(raw[:], acc)
                q2 = scratch.tile([128, TBS], BF16, tag="q2", bufs=10,
                                  name=f"q2_{tb}_{dt}")
                if i % 2 == 0:
                    nc.gpsimd.tensor_mul(q2[:], raw[:], raw[:])
                else:
                    nc.vector.tensor_mul(q2[:], raw[:], raw[:])
                raws[dt] = raw
                q2s[dt] = q2

            # per-tail thunks: ssq matmul (PE, tag pH) + sqrt direct from
            # PSUM + recip + broadcast + stream-shuffle rope (k first).
            tail_dts = [HG, 0, 1, 2, 3]

            def make_tail(i, dt, tb=tb, tsl=tsl, raws=raws, q2s=q2s):
                def emit():
                    ssq = psum.tile([1, TBS], F32, tag="pH",
                                    bufs=1, name=f"ssq_{tb}_{dt}")
                    nc.tensor.matmul(ssq[:], onesb_col[:], q2s[dt][:],
                                     start=True, stop=True)
                    rst = rows.tile([1, TBS], F32, tag="rst", bufs=4,
                                    name=f"rst_{tb}_{dt}")
                    nc.scalar.activation(rst[:], ssq[:],
                                         mybir.ActivationFunctionType.Sqrt,
                                         scale=1.0 / D, bias=eps1[:])
                    rstr = rows.tile([1, TBS], F32, tag="rstr", bufs=4,
                                     name=f"rstr_{tb}_{dt}")
                    nc.vector.reciprocal_approx_fast(out=rstr[:], in_=rst[:])
                    raw = raws[dt]
                    rstdb = scratch.tile([128, TBS], F32, tag="bcast",
                                         bufs=6, name=f"rstdb_{tb}_{dt}")
                    nc.gpsimd.partition_broadcast(rstdb[:], rstr[:])
                    if not skip_w:
                        w_ap = qnw_sb if dt < HG else knw_sb
                        nc.gpsimd.tensor_scalar_mul(rstdb[:], rstdb[:],
                                                    w_ap[:])
                    xsw = scratch.tile([128, TBS], BF16, tag="xsw", bufs=4,
                                       name=f"xsw_{tb}_{dt}")
                    nc.vector.stream_shuffle(xsw[:], raw[:], SWAP_MASK)
                    tmp = scratch.tile([128, TBS], BF16, tag="tmp", bufs=4,
                                       name=f"tmp_{tb}_{dt}")
                    nc.gpsimd.tensor_mul(tmp[:], raw[:], cos_sb[:, tsl])
                    sv = scratch.tile([128, TBS], BF16, tag="sv", bufs=4,
                                      name=f"sv_{tb}_{dt}")
                    nc.vector.tensor_mul(sv[:], xsw[:], sin_sb[:, tsl])
                    qro = scratch.tile([128, TBS], BF16, tag="qro", bufs=4,
                                       name=f"qro_{tb}_{dt}")
                    nc.vector.tensor_add(qro[:], tmp[:], sv[:])
                    dest = qT[:, dt, tsl] if dt < HG else kT[:, tsl]
                    nc.vector.tensor_mul(dest, qro[:], rstdb[:])
                return emit

            for i, dt in enumerate(tail_dts):
                pending_pe.append(make_tail(i, dt))

        flush_pe(1)   # tb3 k-tail: kT cols 1536.. needed from QK kt=12

        # ctx^T per head
        ctxT = [big.tile([D, S], MM_DT, tag=f"ctx{h}", name=f"ctxT_{h}")
                for h in range(HG)]

        # wo loads overlap the first attention blocks ("bigw" frees after
        # the last projection matmul)
        wo_sb = big.tile([128, HG, HID], MM_DT, tag="bigw")
        for ct in range(HG):
            nc.scalar.dma_start(out=wo_sb[:, ct, :],
                                in_=wo[ct * 128:(ct + 1) * 128, :])

        # ---- phase B: attention (qb-major) with Wo folded in ----
        # double-buffered ctx accumulator: halves of one 2-bank pair tile
        ctx2 = psum.tile([128, 2, TBS], F32, tag="pE", name="ctx2")

        pending_wo = []

        def emit_wo(qb):
            thunks = []
            for tt in range(qb * (TBS // 128), (qb + 1) * (TBS // 128)):
                for hc in range(HID // TBS):
                    def thunk(tt=tt, hc=hc):
                        o_ps = psum.tile([128, TBS], F32, tag="pH",
                                         name=f"o_{tt}_{hc}")
                        for ct in range(HG):
                            nc.tensor.matmul(
                                o_ps[:],
                                ctxT[ct][:, tt * 128:(tt + 1) * 128],
                                wo_sb[:, ct, hc * TBS:(hc + 1) * TBS],
                                start=(ct == 0), stop=(ct == HG - 1))
                        o_sb = outp.tile([128, TBS], F32, tag="osb",
                                         name=f"osb_{tt}_{hc}")
                        nc.scalar.copy(o_sb[:], o_ps[:])
                        nc.sync.dma_start(
                            out=out[tt * 128:(tt + 1) * 128,
                                    hc * TBS:(hc + 1) * TBS],
                            in_=o_sb[:])
                    thunks.append(thunk)
            return thunks

        def flush_wo(k):
            for _ in range(min(k, len(pending_wo))):
                pending_wo.pop(0)()

        # cross-block AV stagger; entries: (kt, e_ap, ctx_ap, blk_i)
        pend = []
        norm_jobs = {}

        def flush_av():
            kt0, e0, c_ps, bi = pend.pop(0)
            nc.tensor.matmul(c_ps, v_sb[:, kt0, :], e0,
                             start=(kt0 == 0), stop=(kt0 == KT - 1))
            if kt0 == KT - 1 and bi in norm_jobs:
                norm_jobs.pop(bi)()

        for qb in range(QB):
            qsl = slice(qb * TBS, (qb + 1) * TBS)
            for h in range(HG):
                blk_i = qb * HG + h
                ctx_ps = ctx2[:, blk_i % 2, :]
                eacc = scratch.tile([128, 2, TBS], BF16, tag="eacc", bufs=2,
                                    name=f"eacc_{h}_{qb}")
                pair0 = None
                epair = None
                s_pair = None

                for kt in range(KT):
                    if kt % 2 == 0:
                        s_pair = psum.tile([128, 2, TBS], F32,
                                           tag=["pA", "pC"][(kt // 2) % 2],
                                           name=f"s_{h}_{qb}_{kt}")
                        epair = blk.tile([128, 2, TBS], BF16, tag="blk",
                                         bufs=4, name=f"e_{h}_{qb}_{kt}")
                    nc.tensor.matmul(s_pair[:, kt % 2, :],
                                     kT[:, kt * 128:(kt + 1) * 128],
                                     qT[:, h, qsl], start=True, stop=True)
                    if kt % 2 == 1:
                        # one exp + one accumulator add per key-tile pair
                        nc.scalar.activation(epair[:], s_pair[:],
                                             mybir.ActivationFunctionType.Exp,
                                             scale=SCALE)
                        if kt == 1:
                            pair0 = epair
                        elif kt == 3:
                            nc.vector.tensor_add(eacc[:], pair0[:], epair[:])
                        else:
                            nc.vector.tensor_add(eacc[:], eacc[:], epair[:])
                        pend.append((kt - 1, epair[:, 0, :], ctx_ps, blk_i))
                        pend.append((kt, epair[:, 1, :], ctx_ps, blk_i))
                    while len(pend) > STAGGER:
                        flush_av()
                    # tb3 q-tails: batched at one point (2 ACT table loads)
                    if blk_i == 2 and kt == 6:
                        flush_pe(4)
                    # spread Wo: one output tile (4 matmuls) per slot
                    if qb > 0 and kt % 2 == 1:
                        if (h == 0 and kt >= 9) or h == 1 or \
                           (h == 2 and kt <= 7):
                            flush_wo(1)

                def norm_job(h=h, qb=qb, qsl=qsl, ctx_ps=ctx_ps, eacc=eacc):
                    sum_ps = psum.tile([1, TBS], F32, tag="pG", bufs=1,
                                       name=f"sum_{h}_{qb}")
                    nc.tensor.matmul(sum_ps[:], onesb_col[:], eacc[:, 0, :],
                                     start=True, stop=False)
                    nc.tensor.matmul(sum_ps[:], onesb_col[:], eacc[:, 1, :],
                                     start=False, stop=True)
                    recip = rows.tile([1, TBS], F32, tag="recip",
                                      name=f"recip_{h}_{qb}")
                    nc.vector.reciprocal_approx_fast(out=recip[:],
                                                     in_=sum_ps[:])
                    recipb = scratch.tile([128, TBS], F32, tag="bcast",
                                          bufs=6, name=f"recipb_{h}_{qb}")
                    nc.gpsimd.partition_broadcast(recipb[:], recip[:])
                    nc.vector.tensor_mul(ctxT[h][:, qsl], ctx_ps, recipb[:])
                norm_jobs[blk_i] = norm_job
            pending_wo.extend(emit_wo(qb))

        while pend:
            flush_av()
        for i in sorted(list(norm_jobs)):
            norm_jobs.pop(i)()
        flush_wo(len(pending_wo))

    nc.compile()
    return nc


def _prep_inputs(hidden_states, positions, Wq, Wk, Wv, Wo, q_norm_w, k_norm_w):
    hidden_states = np.asarray(hidden_states, dtype=np.float32)
    positions = np.asarray(positions)
    Wq = np.asarray(Wq, dtype=np.float32)
    Wk = np.asarray(Wk, dtype=np.float32)
    Wv = np.asarray(Wv, dtype=np.float32)
    Wo = np.asarray(Wo, dtype=np.float32)
    q_norm_w = np.asarray(q_norm_w, dtype=np.float32)
    k_norm_w = np.asarray(k_norm_w, dtype=np.float32)

    import ml_dtypes

    # head-dim permutation: pair (j, j+64) -> partitions (2j, 2j+1)
    perm = np.empty(D, dtype=np.int64)
    perm[0::2] = np.arange(HALF)
    perm[1::2] = np.arange(HALF) + HALF

    # permute projection output columns per head
    Wq_p = Wq.reshape(HID, NH, D)[:, :, perm].reshape(HID, NH * D)
    Wk_p = Wk.reshape(HID, NKV, D)[:, :, perm].reshape(HID, NKV * D)
    qnw_p = q_norm_w[perm]
    knw_p = k_norm_w[perm]

    inv_freq = THETA ** (-np.arange(HALF, dtype=np.float32) / HALF)
    in_maps = []
    for c in range(DP * TP):
        b, g = divmod(c, TP)
        freqs = positions[b].astype(np.float32)[:, None] * inv_freq[None, :]
        cos = np.cos(freqs).T.astype(np.float32)      # [64, S]
        sin = np.sin(freqs).T.astype(np.float32)
        # per-pair layout: row 2j/2j+1 both carry cos_j; sin row 2j is
        # -sin_j (even gets -x_odd*sin) and row 2j+1 is +sin_j
        cos2 = np.empty((D, S), dtype=np.float32)
        sin2 = np.empty((D, S), dtype=np.float32)
        cos2[0::2] = cos
        cos2[1::2] = cos
        sin2[0::2] = -sin
        sin2[1::2] = sin
        in_maps.append({
            "xt": np.ascontiguousarray(hidden_states[b].T).astype(PROJ_NP),
            "wq": np.ascontiguousarray(Wq_p[:, g * DQ:(g + 1) * DQ]).astype(PROJ_NP),
            "wk": np.ascontiguousarray(Wk_p[:, g * D:(g + 1) * D]).astype(PROJ_NP),
            "wv": np.ascontiguousarray(Wv[:, g * D:(g + 1) * D]).astype(PROJ_NP),
            "wo": np.ascontiguousarray(Wo[g * DQ:(g + 1) * DQ, :]).astype(MM_NP),
            "cos2": np.ascontiguousarray(cos2).astype(ml_dtypes.bfloat16),
            "sin2": np.ascontiguousarray(sin2).astype(ml_dtypes.bfloat16),
            "qnw": np.ascontiguousarray(qnw_p[:, None]),
            "knw": np.ascontiguousarray(knw_p[:, None]),
            "onesb": np.ones((128, 1), dtype=ml_dtypes.bfloat16),
        })
    return in_maps


def _run(inputs, trace=False):
    skip_w = bool(np.allclose(inputs["q_norm_w"], 1.0)
                  and np.allclose(inputs["k_norm_w"], 1.0))
    key = ("nc", skip_w)
    if key not in _cache:
        _cache[key] = _build(skip_w)
    nc = _cache[key]
    in_maps = _prep_inputs(**inputs)
    res = run_bass_kernel_spmd(nc, in_maps, core_ids=list(range(DP * TP)),
                               trace=trace)
    out = np.zeros((B, S, HID), dtype=np.float32)
    for c in range(DP * TP):
        out[c // TP] += res.results[c]["out"]
    return out, res


def kernel(**inputs):
    out, _ = _run(inputs, trace=False)
    return out


# revision 10
# speedup vs baseline: 1.0478x; 1.0478x over previous
"""Trainium2 Bass kernel for nn_DFlashAttention_43774306681111.

Full-attention transformer block: QKV projection + per-head RMSNorm + neox
RoPE + GQA softmax attention (non-causal) + output projection.

Sharding (8 cores): 2-way data parallel over batch x 4-way tensor parallel
over heads. Core c handles batch c//4 and head group g=c%4 (q heads
4g..4g+3, kv head g). Each core computes a partial output [S, HID]
(its heads' contribution through Wo); the host sums the 4 partials per
batch. No device collectives.

Device layout: activations kept transposed ([dim, token], dim on
partitions) so every matmul contracts on the partition axis:
  Q^T = Wq_tile^T @ X^T          (stationary Wq tile, moving X^T tile)
  S^T[k,q] = K^T_tile^T @ Q^T    (contraction d=128, one matmul per tile)
  softmax over k (= partitions): one exp per KEY-TILE PAIR ([128,1024]
    ACT instruction over a 2-bank psum tile, amortizing the fixed
    activation overhead); the denominator is accumulated with bf16 DVE
    adds over exp pairs and reduced with two ones-vector matmuls per
    block (instead of one per key tile).
  ctx^T[d,q] = V_tile^T @ expS^T (V stationary [k_tok, d])
  out[tok,hid] = ctxT_tile^T @ Wo
V is transposed [d,tok]->[tok,d] by the DMA xbar (dma_start_transpose,
one per token block) -- no PE transposes, no psum evacuations for it.

RoPE: the head dims are PERMUTED host-side (Wq/Wk columns, cos/sin
tables, norm weights) so the rotation pair (i, i+64) sits on adjacent
partitions (2i, 2i+1). The half-swap is then a single DVE stream_shuffle
(even<->odd within each 32-partition quadrant) instead of two SBUF->SBUF
DMAs. Scores/outputs are unchanged because QK contracts over the (same)
permutation of both q and k, and v/ctx/Wo are untouched.

PSUM budget (8 banks): three 2-bank pair tiles (pA: phase-A acc q0/q1
then QK even pairs; pC: acc q2/q3 then QK odd pairs; pE: acc k/v then
the double-buffered ctx accumulator) + pG (softmax sums) + pH (rmsnorm
ssq + Wo output tiles).

Engine discipline: ACT runs ONLY exp in the attention phase (plus the
batched tb3 q-tail sqrts at one point and the Wo psum copies, which
share Exp's table set) so its table never thrashes. All other PSUM
evacuations go to DVE. Wo matmuls are spread one output tile (4
matmuls) per slot through the next q-block's PE stream so exp never
starves behind a long Wo burst.
"""
import numpy as np
from contextlib import ExitStack

import concourse.bass as bass
import concourse.tile as tile
from concourse import bacc, mybir
from concourse.bass_utils import run_bass_kernel_spmd

B, S, HID = 2, 2048, 2048
NH, NKV, D = 16, 4, 128
EPS = 1e-6
THETA = 1000000.0
SCALE = D ** -0.5

TP = 4                 # tensor-parallel groups (heads)
DP = 2                 # data-parallel over batch
HG = NH // TP          # q heads per core = 4
DQ = HG * D            # 512 q-proj cols per core
HALF = D // 2          # 64

F32 = mybir.dt.float32
F32R = mybir.dt.float32r
BF16 = mybir.dt.bfloat16
FP16 = mybir.dt.float16

MM_DT = F32R           # ctx / Wo matmul operand dtype
MM_NP = np.float32
PROJ_DT = FP16         # projection operand dtype (halves phase A DMA)
PROJ_NP = np.float16

HT = HID // 128        # 16 hid tiles
TBS = 512              # token block size
NTB = S // TBS         # 4 token blocks
KT = S // 128          # 16 key tiles
QB = S // TBS          # 4 query blocks
NDT = HG + 2           # 6 projection outputs: q0..q3, k, v^T

STAGGER = 5            # AV matmul lag behind QK/exp

SWAP_MASK = [i ^ 1 for i in range(32)]   # even<->odd partition swap

_cache = {}


def _build(skip_w=False):
    nc = bacc.Bacc(None, target_bir_lowering=False, debug=False)

    xt = nc.dram_tensor("xt", [HID, S], PROJ_DT, kind="ExternalInput")
    wq = nc.dram_tensor("wq", [HID, DQ], PROJ_DT, kind="ExternalInput")
    wk = nc.dram_tensor("wk", [HID, D], PROJ_DT, kind="ExternalInput")
    wv = nc.dram_tensor("wv", [HID, D], PROJ_DT, kind="ExternalInput")
    wo = nc.dram_tensor("wo", [DQ, HID], MM_DT, kind="ExternalInput")
    cos2 = nc.dram_tensor("cos2", [D, S], BF16, kind="ExternalInput")
    sin2 = nc.dram_tensor("sin2", [D, S], BF16, kind="ExternalInput")
    qnw = nc.dram_tensor("qnw", [D, 1], F32, kind="ExternalInput")
    knw = nc.dram_tensor("knw", [D, 1], F32, kind="ExternalInput")
    onesb_d = nc.dram_tensor("onesb", [128, 1], BF16, kind="ExternalInput")
    out = nc.dram_tensor("out", [S, HID], F32, kind="ExternalOutput")

    with tile.TileContext(nc) as tc, ExitStack() as ctx:
        const = ctx.enter_context(tc.tile_pool(name="const", bufs=1))
        big = ctx.enter_context(tc.tile_pool(name="big", bufs=1))
        blk = ctx.enter_context(tc.tile_pool(name="blk", bufs=8))
        outp = ctx.enter_context(tc.tile_pool(name="outp", bufs=3))
        scratch = ctx.enter_context(tc.tile_pool(name="scratch", bufs=2))
        rows = ctx.enter_context(tc.tile_pool(name="rows", bufs=2))
        psum = ctx.enter_context(tc.tile_pool(name="psum", bufs=1, space="PSUM"))

        # ---- constants ----
        onesb_col = const.tile([128, 1], BF16)
        nc.scalar.dma_start(out=onesb_col[:], in_=onesb_d[:])
        eps1 = const.tile([1, 1], F32)
        nc.vector.memset(eps1, EPS)
        qnw_sb = const.tile([D, 1], F32)
        nc.scalar.dma_start(out=qnw_sb[:], in_=qnw[:])
        knw_sb = const.tile([D, 1], F32)
        nc.scalar.dma_start(out=knw_sb[:], in_=knw[:])

        # ---- resident weights / big activations (tag-shared slots) ----
        wq_sb = big.tile([128, HT, DQ], PROJ_DT, tag="bigw")
        wk_sb = big.tile([128, HT, D], PROJ_DT, tag="wk")
        wv_sb = big.tile([128, HT, D], PROJ_DT, tag="wv")
        cos_sb = big.tile([D, S], BF16, tag="cosb")
        sin_sb = big.tile([D, S], BF16, tag="sinb")

        qT = big.tile([D, HG, S], BF16, tag="qT")        # Q^T per head
        kT = big.tile([D, S], BF16, tag="kT")            # K^T
        vT = big.tile([D, S], BF16, tag="vT")            # V^T (pre-transpose)
        v_sb = big.tile([128, KT, D], BF16, tag="v")     # V [tok, d] tiles

        def stationary(ht, dt):
            if dt < HG:
                return wq_sb[:, ht, dt * D:(dt + 1) * D]
            if dt == HG:
                return wk_sb[:, ht, :]
            return wv_sb[:, ht, :]

        # Deferred PE work from the rmsnorm tails: one ssq matmul + sqrt +
        # recip + broadcast + rope chain per projection output. Flushed at
        # spread-out points of the later PE stream.
        pending_pe = []

        def flush_pe(k=1):
            for _ in range(min(k, len(pending_pe))):
                pending_pe.pop(0)()

        # ---- phase A: projections ----
        for tb in range(NTB):
            tsl = slice(tb * TBS, (tb + 1) * TBS)
            pairs = [psum.tile([128, 2, TBS], F32, tag=t,
                               name=f"accp_{tb}_{t}")
                     for t in ("pA", "pC", "pE")]
            accs = [pairs[dt // 2][:, dt % 2, :] for dt in range(NDT)]
            for ht in range(HT):
                if tb == 0:
                    hsl = slice(ht * 128, (ht + 1) * 128)
                    nc.scalar.dma_start(out=wq_sb[:, ht, :], in_=wq[hsl, :])
                    nc.scalar.dma_start(out=wk_sb[:, ht, :], in_=wk[hsl, :])
                    nc.scalar.dma_start(out=wv_sb[:, ht, :], in_=wv[hsl, :])
                if ht == 1:
                    nc.scalar.dma_start(out=cos_sb[:, tsl], in_=cos2[:, tsl])
                    nc.scalar.dma_start(out=sin_sb[:, tsl], in_=sin2[:, tsl])
                if ht in (4, 6, 8, 10, 12):
                    flush_pe(1)  # previous tb's rmsnorm/rope tails
                xt_t = blk.tile([128, TBS], PROJ_DT, tag="xt", bufs=12,
                                name=f"xt_{tb}_{ht}")
                nc.sync.dma_start(out=xt_t[:], in_=xt[ht * 128:(ht + 1) * 128, tsl])
                for dt in range(NDT):
                    nc.tensor.matmul(accs[dt], stationary(ht, dt), xt_t[:],
                                     start=(ht == 0), stop=(ht == HT - 1))

            # ---- end of tb: evacuate psums fast (k first: its tail ssq
            # flushes earliest), then queue tail work ----
            raws = {}
            q2s = {}
            for i, dt in enumerate([HG, 0, 1, NDT - 1, 2, 3]):
                acc = accs[dt]
                if dt == NDT - 1:
                    # v^T: evac, then DMA-transpose this tb's chunk into
                    # v_sb [tok, d] on an idle DMA queue (no PE transposes)
                    nc.vector.tensor_copy(vT[:, tsl], acc)
                    nc.sync.dma_start_transpose(
                        v_sb[:, tb * (TBS // 128):(tb + 1) * (TBS // 128), :],
                        vT[:, tsl])
                    continue
                raw = scratch.tile([128, TBS], BF16, tag="raw", bufs=10,
                                   name=f"raw_{tb}_{dt}")
                nc.vector.tensor_copy(raw[:], acc)
                q2 = scratch.tile([128, TBS], BF16, tag="q2", bufs=10,
                                  name=f"q2_{tb}_{dt}")
                nc.gpsimd.tensor_mul(q2[:], raw[:], raw[:])
                raws[dt] = raw
                q2s[dt] = q2

            # per-tail thunks: ssq matmul (PE, tag pH) + sqrt direct from
            # PSUM + recip + broadcast + stream-shuffle rope (k first).
            tail_dts = [HG, 0, 1, 2, 3]

            def make_tail(i, dt, tb=tb, tsl=tsl, raws=raws, q2s=q2s):
                def emit():
                    ssq = psum.tile([1, TBS], F32, tag="pH",
                                    bufs=1, name=f"ssq_{tb}_{dt}")
                    nc.tensor.matmul(ssq[:], onesb_col[:], q2s[dt][:],
                                     start=True, stop=True)
                    rst = rows.tile([1, TBS], F32, tag="rst", bufs=4,
                                    name=f"rst_{tb}_{dt}")
                    nc.scalar.activation(rst[:], ssq[:],
                                         mybir.ActivationFunctionType.Sqrt,
                                         scale=1.0 / D, bias=eps1[:])
                    rstr = rows.tile([1, TBS], F32, tag="rstr", bufs=4,
                                     name=f"rstr_{tb}_{dt}")
                    nc.vector.reciprocal_approx_fast(out=rstr[:], in_=rst[:])
                    raw = raws[dt]
                    rstdb = scratch.tile([128, TBS], F32, tag="bcast",
                                         bufs=6, name=f"rstdb_{tb}_{dt}")
                    nc.gpsimd.partition_broadcast(rstdb[:], rstr[:])
                    xsw = scratch.tile([128, TBS], BF16, tag="xsw", bufs=4,
                                       name=f"xsw_{tb}_{dt}")
                    nc.vector.stream_shuffle(xsw[:], raw[:], SWAP_MASK)
                    tmp = scratch.tile([128, TBS], BF16, tag="tmp", bufs=4,
                                       name=f"tmp_{tb}_{dt}")
                    nc.gpsimd.tensor_mul(tmp[:], raw[:], cos_sb[:, tsl])
                    sv = scratch.tile([128, TBS], BF16, tag="sv", bufs=4,
                                      name=f"sv_{tb}_{dt}")
                    nc.vector.tensor_mul(sv[:], xsw[:], sin_sb[:, tsl])
                    qro = scratch.tile([128, TBS], BF16, tag="qro", bufs=4,
                                       name=f"qro_{tb}_{dt}")
                    nc.vector.tensor_add(qro[:], tmp[:], sv[:])
                    dest = qT[:, dt, tsl] if dt < HG else kT[:, tsl]
                    nc.vector.tensor_mul(dest, qro[:], rstdb[:])
                    if not skip_w:
                        w_ap = qnw_sb if dt < HG else knw_sb
                        nc.vector.tensor_scalar_mul(dest, dest, w_ap[:])
                return emit

            for i, dt in enumerate(tail_dts):
                pending_pe.append(make_tail(i, dt))

        flush_pe(1)   # tb3 k-tail: kT cols 1536.. needed from QK kt=12

        # ctx^T per head
        ctxT = [big.tile([D, S], MM_DT, tag=f"ctx{h}", name=f"ctxT_{h}")
                for h in range(HG)]

        # wo loads overlap the first attention blocks ("bigw" frees after
        # the last projection matmul)
        wo_sb = big.tile([128, HG, HID], MM_DT, tag="bigw")
        for ct in range(HG):
            nc.scalar.dma_start(out=wo_sb[:, ct, :],
                                in_=wo[ct * 128:(ct + 1) * 128, :])

        # ---- phase B: attention (qb-major) with Wo folded in ----
        # double-buffered ctx accumulator: halves of one 2-bank pair tile
        ctx2 = psum.tile([128, 2, TBS], F32, tag="pE", name="ctx2")

        pending_wo = []

        def emit_wo(qb):
            thunks = []
            for tt in range(qb * (TBS // 128), (qb + 1) * (TBS // 128)):
                for hc in range(HID // TBS):
                    def thunk(tag, evac, tt=tt, hc=hc):
                        o_ps = psum.tile([128, TBS], F32, tag=tag,
                                         name=f"o_{tt}_{hc}")
                        for ct in range(HG):
                            nc.tensor.matmul(
                                o_ps[:],
                                ctxT[ct][:, tt * 128:(tt + 1) * 128],
                                wo_sb[:, ct, hc * TBS:(hc + 1) * TBS],
                                start=(ct == 0), stop=(ct == HG - 1))
                        o_sb = outp.tile([128, TBS], F32, tag="osb",
                                         name=f"osb_{tt}_{hc}")
                        if evac == 0:
                            nc.scalar.copy(o_sb[:], o_ps[:])
                        else:
                            nc.vector.tensor_copy(o_sb[:], o_ps[:])
                        nc.sync.dma_start(
                            out=out[tt * 128:(tt + 1) * 128,
                                    hc * TBS:(hc + 1) * TBS],
                            in_=o_sb[:])
                    thunks.append(thunk)
            return thunks

        def flush_wo(k, drain=False):
            for j in range(min(k, len(pending_wo))):
                # mid-attention: single bank (pH), ACT evac. Drain: alternate
                # pG/pH banks and ACT/DVE evacs so tiles pipeline.
                if drain:
                    pending_wo.pop(0)(["pH", "pG"][j % 2], j % 2)
                else:
                    pending_wo.pop(0)("pH", 0)

        # cross-block AV stagger; entries: (kt, e_ap, ctx_ap, blk_i)
        pend = []
        norm_jobs = {}

        def flush_av():
            kt0, e0, c_ps, bi = pend.pop(0)
            nc.tensor.matmul(c_ps, v_sb[:, kt0, :], e0,
                             start=(kt0 == 0), stop=(kt0 == KT - 1))
            if kt0 == KT - 1 and bi in norm_jobs:
                norm_jobs.pop(bi)()

        for qb in range(QB):
            qsl = slice(qb * TBS, (qb + 1) * TBS)
            for h in range(HG):
                blk_i = qb * HG + h
                ctx_ps = ctx2[:, blk_i % 2, :]
                eacc = scratch.tile([128, 2, TBS], BF16, tag="eacc", bufs=2,
                                    name=f"eacc_{h}_{qb}")
                pair0 = None
                epair = None
                s_pair = None

                for kt in range(KT):
                    if kt % 2 == 0:
                        s_pair = psum.tile([128, 2, TBS], F32,
                                           tag=["pA", "pC"][(kt // 2) % 2],
                                           name=f"s_{h}_{qb}_{kt}")
                        epair = blk.tile([128, 2, TBS], BF16, tag="blk",
                                         bufs=4, name=f"e_{h}_{qb}_{kt}")
                    nc.tensor.matmul(s_pair[:, kt % 2, :],
                                     kT[:, kt * 128:(kt + 1) * 128],
                                     qT[:, h, qsl], start=True, stop=True)
                    if kt % 2 == 1:
                        # one exp + one accumulator add per key-tile pair
                        nc.scalar.activation(epair[:], s_pair[:],
                                             mybir.ActivationFunctionType.Exp,
                                             scale=SCALE)
                        if kt == 1:
                            pair0 = epair
                        elif kt == 3:
                            nc.vector.tensor_add(eacc[:], pair0[:], epair[:])
                        else:
                            nc.vector.tensor_add(eacc[:], eacc[:], epair[:])
                        pend.append((kt - 1, epair[:, 0, :], ctx_ps, blk_i))
                        pend.append((kt, epair[:, 1, :], ctx_ps, blk_i))
                    while len(pend) > STAGGER:
                        flush_av()
                    # tb3 q-tails: two per early block (batched sqrts)
                    if blk_i in (1, 2) and kt == 6:
                        flush_pe(2)
                    # spread Wo: one output tile (4 matmuls) per slot
                    if qb > 0 and kt % 2 == 1:
                        if (h == 0 and kt >= 9) or h == 1 or \
                           (h == 2 and kt <= 7):
                            flush_wo(1)

                def norm_job(h=h, qb=qb, qsl=qsl, ctx_ps=ctx_ps, eacc=eacc):
                    sum_ps = psum.tile([1, TBS], F32, tag="pG", bufs=1,
                                       name=f"sum_{h}_{qb}")
                    nc.tensor.matmul(sum_ps[:], onesb_col[:], eacc[:, 0, :],
                                     start=True, stop=False)
                    nc.tensor.matmul(sum_ps[:], onesb_col[:], eacc[:, 1, :],
                                     start=False, stop=True)
                    recip = rows.tile([1, TBS], F32, tag="recip",
                                      name=f"recip_{h}_{qb}")
                    nc.vector.reciprocal_approx_fast(out=recip[:],
                                                     in_=sum_ps[:])
                    recipb = scratch.tile([128, TBS], F32, tag="bcast",
                                          bufs=6, name=f"recipb_{h}_{qb}")
                    nc.gpsimd.partition_broadcast(recipb[:], recip[:])
                    nc.vector.tensor_mul(ctxT[h][:, qsl], ctx_ps, recipb[:])
                norm_jobs[blk_i] = norm_job
            pending_wo.extend(emit_wo(qb))

        while pend:
            flush_av()
        for i in sorted(list(norm_jobs)):
            norm_jobs.pop(i)()
        flush_wo(len(pending_wo), drain=True)

    nc.compile()
    return nc


def _prep_inputs(hidden_states, positions, Wq, Wk, Wv, Wo, q_norm_w, k_norm_w):
    hidden_states = np.asarray(hidden_states, dtype=np.float32)
    positions = np.asarray(positions)
    Wq = np.asarray(Wq, dtype=np.float32)
    Wk = np.asarray(Wk, dtype=np.float32)
    Wv = np.asarray(Wv, dtype=np.float32)
    Wo = np.asarray(Wo, dtype=np.float32)
    q_norm_w = np.asarray(q_norm_w, dtype=np.float32)
    k_norm_w = np.asarray(k_norm_w, dtype=np.float32)

    import ml_dtypes

    # head-dim permutation: pair (j, j+64) -> partitions (2j, 2j+1)
    perm = np.empty(D, dtype=np.int64)
    perm[0::2] = np.arange(HALF)
    perm[1::2] = np.arange(HALF) + HALF

    # permute projection output columns per head
    Wq_p = Wq.reshape(HID, NH, D)[:, :, perm].reshape(HID, NH * D)
    Wk_p = Wk.reshape(HID, NKV, D)[:, :, perm].reshape(HID, NKV * D)
    qnw_p = q_norm_w[perm]
    knw_p = k_norm_w[perm]

    inv_freq = THETA ** (-np.arange(HALF, dtype=np.float32) / HALF)
    in_maps = []
    for c in range(DP * TP):
        b, g = divmod(c, TP)
        freqs = positions[b].astype(np.float32)[:, None] * inv_freq[None, :]
        cos = np.cos(freqs).T.astype(np.float32)      # [64, S]
        sin = np.sin(freqs).T.astype(np.float32)
        # per-pair layout: row 2j/2j+1 both carry cos_j; sin row 2j is
        # -sin_j (even gets -x_odd*sin) and row 2j+1 is +sin_j
        cos2 = np.empty((D, S), dtype=np.float32)
        sin2 = np.empty((D, S), dtype=np.float32)
        cos2[0::2] = cos
        cos2[1::2] = cos
        sin2[0::2] = -sin
        sin2[1::2] = sin
        in_maps.append({
            "xt": np.ascontiguousarray(hidden_states[b].T).astype(PROJ_NP),
            "wq": np.ascontiguousarray(Wq_p[:, g * DQ:(g + 1) * DQ]).astype(PROJ_NP),
            "wk": np.ascontiguousarray(Wk_p[:, g * D:(g + 1) * D]).astype(PROJ_NP),
            "wv": np.ascontiguousarray(Wv[:, g * D:(g + 1) * D]).astype(PROJ_NP),
            "wo": np.ascontiguousarray(Wo[g * DQ:(g + 1) * DQ, :]).astype(MM_NP),
            "cos2": np.ascontiguousarray(cos2).astype(ml_dtypes.bfloat16),
            "sin2": np.ascontiguousarray(sin2).astype(ml_dtypes.bfloat16),
            "qnw": np.ascontiguousarray(qnw_p[:, None]),
            "knw": np.ascontiguousarray(knw_p[:, None]),
            "onesb": np.ones((128, 1), dtype=ml_dtypes.bfloat16),
        })
    return in_maps


def _run(inputs, trace=False):
    skip_w = bool(np.allclose(inputs["q_norm_w"], 1.0)
                  and np.allclose(inputs["k_norm_w"], 1.0))
    key = ("nc", skip_w)
    if key not in _cache:
        _cache[key] = _build(skip_w)
    nc = _cache[key]
    in_maps = _prep_inputs(**inputs)
    res = run_bass_kernel_spmd(nc, in_maps, core_ids=list(range(DP * TP)),
                               trace=trace)
    out = np.zeros((B, S, HID), dtype=np.float32)
    for c in range(DP * TP):
        out[c // TP] += res.results[c]["out"]
    return out, res


def kernel(**inputs):
    out, _ = _run(inputs, trace=False)
    return out


# revision 12
# speedup vs baseline: 1.1880x; 1.1338x over previous
"""Trainium2 Bass kernel for nn_DFlashAttention_43774306681111.

Full-attention transformer block: QKV projection + per-head RMSNorm + neox
RoPE + GQA softmax attention (non-causal) + output projection.

Sharding (8 cores): 2-way data parallel over batch x 4-way tensor parallel
over heads. Core c handles batch c//4 and head group g=c%4 (q heads
4g..4g+3, kv head g). Each core computes a partial output [S, HID]
(its heads' contribution through Wo); the host sums the 4 partials per
batch. No device collectives.

Device layout: activations kept transposed ([dim, token], dim on
partitions) so every matmul contracts on the partition axis:
  Q^T = Wq_tile^T @ X^T          (stationary Wq tile, moving X^T tile)
  S^T[k,q] = K^T_tile^T @ Q^T    (contraction d=128, one matmul per tile)
  softmax over k (= partitions): one exp per KEY-TILE PAIR ([128,1024]
    ACT instruction over a 2-bank psum tile, amortizing the fixed
    activation overhead); the denominator is accumulated with bf16 DVE
    adds over exp pairs and reduced with two ones-vector matmuls per
    block (instead of one per key tile).
  ctx^T[d,q] = V_tile^T @ expS^T (V stationary [k_tok, d])
  out[tok,hid] = ctxT_tile^T @ Wo
V is transposed [d,tok]->[tok,d] by the DMA xbar (dma_start_transpose,
one per token block) -- no PE transposes, no psum evacuations for it.

RoPE: the head dims are PERMUTED host-side (Wq/Wk columns, cos/sin
tables, norm weights) so the rotation pair (i, i+64) sits on adjacent
partitions (2i, 2i+1). The half-swap is then a single DVE stream_shuffle
(even<->odd within each 32-partition quadrant) instead of two SBUF->SBUF
DMAs. Scores/outputs are unchanged because QK contracts over the (same)
permutation of both q and k, and v/ctx/Wo are untouched.

PSUM budget (8 banks): three 2-bank pair tiles (pA: phase-A acc q0/q1
then QK even pairs; pC: acc q2/q3 then QK odd pairs; pE: acc k/v then
the double-buffered ctx accumulator) + pG (softmax sums) + pH (rmsnorm
ssq + Wo output tiles).

Engine discipline: ACT runs ONLY exp in the attention phase (plus the
batched tb3 q-tail sqrts at one point and the Wo psum copies, which
share Exp's table set) so its table never thrashes. All other PSUM
evacuations go to DVE. Wo matmuls are spread one output tile (4
matmuls) per slot through the next q-block's PE stream so exp never
starves behind a long Wo burst.
"""
import numpy as np
from contextlib import ExitStack

import concourse.bass as bass
import concourse.tile as tile
from concourse import bacc, mybir
from concourse.bass_utils import run_bass_kernel_spmd

B, S, HID = 2, 2048, 2048
NH, NKV, D = 16, 4, 128
EPS = 1e-6
THETA = 1000000.0
SCALE = D ** -0.5

TP = 4                 # tensor-parallel groups (heads)
DP = 2                 # data-parallel over batch
HG = NH // TP          # q heads per core = 4
DQ = HG * D            # 512 q-proj cols per core
HALF = D // 2          # 64

F32 = mybir.dt.float32
F32R = mybir.dt.float32r
BF16 = mybir.dt.bfloat16
FP16 = mybir.dt.float16

MM_DT = F32R           # ctx / Wo matmul operand dtype
MM_NP = np.float32
PROJ_DT = FP16         # projection operand dtype (halves phase A DMA)
PROJ_NP = np.float16

HT = HID // 128        # 16 hid tiles
TBS = 512              # token block size
NTB = S // TBS         # 4 token blocks
KT = S // 128          # 16 key tiles
QB = S // TBS          # 4 query blocks
NDT = HG + 2           # 6 projection outputs: q0..q3, k, v^T

STAGGER = 5            # AV matmul lag behind QK/exp

SWAP_MASK = [i ^ 1 for i in range(32)]   # even<->odd partition swap

_cache = {}


def _build(skip_w=False):
    nc = bacc.Bacc(None, target_bir_lowering=False, debug=False)

    xt = nc.dram_tensor("xt", [HID, S], PROJ_DT, kind="ExternalInput")
    wq = nc.dram_tensor("wq", [HID, DQ], PROJ_DT, kind="ExternalInput")
    wk = nc.dram_tensor("wk", [HID, D], PROJ_DT, kind="ExternalInput")
    wv = nc.dram_tensor("wv", [HID, D], PROJ_DT, kind="ExternalInput")
    wo = nc.dram_tensor("wo", [DQ, HID], MM_DT, kind="ExternalInput")
    cos2 = nc.dram_tensor("cos2", [D, S], BF16, kind="ExternalInput")
    sin2 = nc.dram_tensor("sin2", [D, S], BF16, kind="ExternalInput")
    qnw = nc.dram_tensor("qnw", [D, 1], F32, kind="ExternalInput")
    knw = nc.dram_tensor("knw", [D, 1], F32, kind="ExternalInput")
    onesb_d = nc.dram_tensor("onesb", [128, 1], BF16, kind="ExternalInput")
    out = nc.dram_tensor("out", [S, HID], F32, kind="ExternalOutput")

    with tile.TileContext(nc) as tc, ExitStack() as ctx:
        const = ctx.enter_context(tc.tile_pool(name="const", bufs=1))
        big = ctx.enter_context(tc.tile_pool(name="big", bufs=1))
        blk = ctx.enter_context(tc.tile_pool(name="blk", bufs=8))
        outp = ctx.enter_context(tc.tile_pool(name="outp", bufs=3))
        scratch = ctx.enter_context(tc.tile_pool(name="scratch", bufs=2))
        rows = ctx.enter_context(tc.tile_pool(name="rows", bufs=2))
        psum = ctx.enter_context(tc.tile_pool(name="psum", bufs=1, space="PSUM"))

        # ---- constants ----
        onesb_col = const.tile([128, 1], BF16)
        nc.scalar.dma_start(out=onesb_col[:], in_=onesb_d[:])
        eps1 = const.tile([1, 1], F32)
        nc.vector.memset(eps1, EPS)
        qnw_sb = const.tile([D, 1], F32)
        nc.scalar.dma_start(out=qnw_sb[:], in_=qnw[:])
        knw_sb = const.tile([D, 1], F32)
        nc.scalar.dma_start(out=knw_sb[:], in_=knw[:])

        # ---- resident weights / big activations (tag-shared slots) ----
        wq_sb = big.tile([128, HT, DQ], PROJ_DT, tag="bigw")
        wk_sb = big.tile([128, HT, D], PROJ_DT, tag="wk")
        wv_sb = big.tile([128, HT, D], PROJ_DT, tag="wv")
        cos_sb = big.tile([D, S], BF16, tag="cosb")
        sin_sb = big.tile([D, S], BF16, tag="sinb")

        qT = big.tile([D, HG, S], BF16, tag="qT")        # Q^T per head
        kT = big.tile([D, S], BF16, tag="kT")            # K^T
        vT = big.tile([D, S], BF16, tag="vT")            # V^T (pre-transpose)
        v_sb = big.tile([128, KT, D], BF16, tag="v")     # V [tok, d] tiles

        def stationary(ht, dt):
            if dt < HG:
                return wq_sb[:, ht, dt * D:(dt + 1) * D]
            if dt == HG:
                return wk_sb[:, ht, :]
            return wv_sb[:, ht, :]

        # Deferred PE work from the rmsnorm tails: one ssq matmul + sqrt +
        # recip + broadcast + rope chain per projection output. Flushed at
        # spread-out points of the later PE stream.
        pending_pe = []

        def flush_pe(k=1):
            for _ in range(min(k, len(pending_pe))):
                pending_pe.pop(0)()

        # ---- phase A: projections ----
        for tb in range(NTB):
            tsl = slice(tb * TBS, (tb + 1) * TBS)
            pairs = [psum.tile([128, 2, TBS], F32, tag=t,
                               name=f"accp_{tb}_{t}")
                     for t in ("pA", "pC", "pE")]
            accs = [pairs[dt // 2][:, dt % 2, :] for dt in range(NDT)]
            for ht in range(HT):
                if tb == 0:
                    hsl = slice(ht * 128, (ht + 1) * 128)
                    nc.scalar.dma_start(out=wq_sb[:, ht, :], in_=wq[hsl, :])
                    nc.scalar.dma_start(out=wk_sb[:, ht, :], in_=wk[hsl, :])
                    nc.scalar.dma_start(out=wv_sb[:, ht, :], in_=wv[hsl, :])
                if ht == 1:
                    nc.scalar.dma_start(out=cos_sb[:, tsl], in_=cos2[:, tsl])
                    nc.scalar.dma_start(out=sin_sb[:, tsl], in_=sin2[:, tsl])
                if ht in (4, 6, 8, 10, 12):
                    flush_pe(1)  # previous tb's rmsnorm/rope tails
                xt_t = blk.tile([128, TBS], PROJ_DT, tag="xt", bufs=12,
                                name=f"xt_{tb}_{ht}")
                nc.sync.dma_start(out=xt_t[:], in_=xt[ht * 128:(ht + 1) * 128, tsl])
                for dt in range(NDT):
                    nc.tensor.matmul(accs[dt], stationary(ht, dt), xt_t[:],
                                     start=(ht == 0), stop=(ht == HT - 1))

            # ---- end of tb: evacuate psums fast (k first: its tail ssq
            # flushes earliest), then queue tail work ----
            raws = {}
            q2s = {}
            for i, dt in enumerate([HG, 0, 1, NDT - 1, 2, 3]):
                acc = accs[dt]
                if dt == NDT - 1:
                    # v^T: evac, then DMA-transpose this tb's chunk into
                    # v_sb [tok, d] on an idle DMA queue (no PE transposes)
                    nc.vector.tensor_copy(vT[:, tsl], acc)
                    nc.sync.dma_start_transpose(
                        v_sb[:, tb * (TBS // 128):(tb + 1) * (TBS // 128), :],
                        vT[:, tsl])
                    continue
                raw = scratch.tile([128, TBS], BF16, tag="raw", bufs=10,
                                   name=f"raw_{tb}_{dt}")
                nc.vector.tensor_copy(raw[:], acc)
                q2 = scratch.tile([128, TBS], BF16, tag="q2", bufs=7,
                                  name=f"q2_{tb}_{dt}")
                nc.gpsimd.tensor_mul(q2[:], raw[:], raw[:])
                raws[dt] = raw
                q2s[dt] = q2

            # Tail work, staged to avoid cross-engine ping-pong: the five
            # ssq matmuls (psum pH) + sqrts land in a shared [1,5,TBS] row
            # tile; reciprocals are batched (k+q0, then q1..q3); the rope
            # chains run DVE-only with one gps broadcast per tail.
            tail_dts = [HG, 0, 1, 2, 3]
            rst5 = rows.tile([1, 5, TBS], F32, tag="rst", bufs=1,
                             name=f"rst5_{tb}")
            rstr5 = rows.tile([1, 5, TBS], F32, tag="rstr", bufs=1,
                              name=f"rstr5_{tb}")

            def emit_ssq(i, dt, tb=tb, rst5=rst5, q2s=q2s):
                ssq = psum.tile([1, TBS], F32, tag="pH",
                                bufs=1, name=f"ssq_{tb}_{dt}")
                nc.tensor.matmul(ssq[:], onesb_col[:], q2s[dt][:],
                                 start=True, stop=True)
                nc.scalar.activation(rst5[0:1, i, :], ssq[:],
                                     mybir.ActivationFunctionType.Sqrt,
                                     scale=1.0 / D, bias=eps1[:])

            def emit_chain(i, dt, tb=tb, tsl=tsl, raws=raws, rstr5=rstr5):
                raw = raws[dt]
                rstdb = scratch.tile([128, TBS], F32, tag="bcast",
                                     bufs=6, name=f"rstdb_{tb}_{dt}")
                nc.gpsimd.partition_broadcast(rstdb[:], rstr5[0:1, i, :])
                xsw = scratch.tile([128, TBS], BF16, tag="xsw", bufs=4,
                                   name=f"xsw_{tb}_{dt}")
                nc.vector.stream_shuffle(xsw[:], raw[:], SWAP_MASK)
                tmp = scratch.tile([128, TBS], BF16, tag="tmp", bufs=4,
                                   name=f"tmp_{tb}_{dt}")
                nc.vector.tensor_mul(tmp[:], raw[:], cos_sb[:, tsl])
                sv = scratch.tile([128, TBS], BF16, tag="sv", bufs=4,
                                  name=f"sv_{tb}_{dt}")
                nc.vector.tensor_mul(sv[:], xsw[:], sin_sb[:, tsl])
                qro = scratch.tile([128, TBS], BF16, tag="qro", bufs=4,
                                   name=f"qro_{tb}_{dt}")
                nc.vector.tensor_add(qro[:], tmp[:], sv[:])
                dest = qT[:, dt, tsl] if dt < HG else kT[:, tsl]
                nc.vector.tensor_mul(dest, qro[:], rstdb[:])
                if not skip_w:
                    w_ap = qnw_sb if dt < HG else knw_sb
                    nc.vector.tensor_scalar_mul(dest, dest, w_ap[:])

            def entry0(tb=tb):
                emit_ssq(0, HG)
            def entry1(tb=tb, rst5=rst5, rstr5=rstr5):
                emit_ssq(1, 0)
                nc.vector.reciprocal_approx_fast(out=rstr5[0:1, 0:2, :],
                                                 in_=rst5[0:1, 0:2, :])
                emit_chain(0, HG)
                emit_chain(1, 0)
            def entry2(tb=tb):
                emit_ssq(2, 1)
            def entry3(tb=tb):
                emit_ssq(3, 2)
            def entry4(tb=tb, rst5=rst5, rstr5=rstr5):
                emit_ssq(4, 3)
                nc.vector.reciprocal_approx_fast(out=rstr5[0:1, 2:5, :],
                                                 in_=rst5[0:1, 2:5, :])
                emit_chain(2, 1)
                emit_chain(3, 2)
                emit_chain(4, 3)

            pending_pe.extend([entry0, entry1, entry2, entry3, entry4])

        flush_pe(2)   # tb3 k+q0 tails: kT cols 1536.. needed from QK kt=12

        # ctx^T per head
        ctxT = [big.tile([D, S], MM_DT, tag=f"ctx{h}", name=f"ctxT_{h}")
                for h in range(HG)]

        # wo loads overlap the first attention blocks ("bigw" frees after
        # the last projection matmul)
        wo_sb = big.tile([128, HG, HID], MM_DT, tag="bigw")
        for ct in range(HG):
            nc.scalar.dma_start(out=wo_sb[:, ct, :],
                                in_=wo[ct * 128:(ct + 1) * 128, :])

        # ---- phase B: attention (qb-major) with Wo folded in ----
        # double-buffered ctx accumulator: halves of one 2-bank pair tile
        ctx2 = psum.tile([128, 2, TBS], F32, tag="pE", name="ctx2")

        pending_wo = []

        def emit_wo(qb):
            thunks = []
            for tt in range(qb * (TBS // 128), (qb + 1) * (TBS // 128)):
                for hc in range(HID // TBS):
                    def thunk(tag, evac, tt=tt, hc=hc):
                        o_ps = psum.tile([128, TBS], F32, tag=tag,
                                         name=f"o_{tt}_{hc}")
                        for ct in range(HG):
                            nc.tensor.matmul(
                                o_ps[:],
                                ctxT[ct][:, tt * 128:(tt + 1) * 128],
                                wo_sb[:, ct, hc * TBS:(hc + 1) * TBS],
                                start=(ct == 0), stop=(ct == HG - 1))
                        o_sb = outp.tile([128, TBS], F32, tag="osb",
                                         name=f"osb_{tt}_{hc}")
                        if evac == 0:
                            nc.scalar.copy(o_sb[:], o_ps[:])
                        else:
                            nc.vector.tensor_copy(o_sb[:], o_ps[:])
                        nc.sync.dma_start(
                            out=out[tt * 128:(tt + 1) * 128,
                                    hc * TBS:(hc + 1) * TBS],
                            in_=o_sb[:])
                    thunks.append(thunk)
            return thunks

        wo_flip = [0]

        def flush_wo(k, drain=False):
            for _ in range(min(k, len(pending_wo))):
                j = wo_flip[0] = wo_flip[0] + 1
                pending_wo.pop(0)(["pH", "pG"][j % 2], j % 2)

        # cross-block AV stagger; entries: (kt, e_ap, ctx_ap, blk_i)
        pend = []
        norm_jobs = {}

        def flush_av():
            kt0, e0, c_ps, bi = pend.pop(0)
            nc.tensor.matmul(c_ps, v_sb[:, kt0, :], e0,
                             start=(kt0 == 0), stop=(kt0 == KT - 1))
            if kt0 == KT - 1 and bi in norm_jobs:
                norm_jobs.pop(bi)()

        for qb in range(QB):
            qsl = slice(qb * TBS, (qb + 1) * TBS)
            for h in range(HG):
                blk_i = qb * HG + h
                ctx_ps = ctx2[:, blk_i % 2, :]
                eacc = scratch.tile([128, 2, TBS], BF16, tag="eacc", bufs=2,
                                    name=f"eacc_{h}_{qb}")
                pair0 = None
                epair = None
                s_pair = None

                for kt in range(KT):
                    if kt % 2 == 0:
                        s_pair = psum.tile([128, 2, TBS], F32,
                                           tag=["pA", "pC"][(kt // 2) % 2],
                                           name=f"s_{h}_{qb}_{kt}")
                        epair = blk.tile([128, 2, TBS], BF16, tag="blk",
                                         bufs=4, name=f"e_{h}_{qb}_{kt}")
                    nc.tensor.matmul(s_pair[:, kt % 2, :],
                                     kT[:, kt * 128:(kt + 1) * 128],
                                     qT[:, h, qsl], start=True, stop=True)
                    if kt % 2 == 1:
                        # one exp + one accumulator add per key-tile pair
                        nc.scalar.activation(epair[:], s_pair[:],
                                             mybir.ActivationFunctionType.Exp,
                                             scale=SCALE)
                        if kt == 1:
                            pair0 = epair
                        elif kt == 3:
                            nc.vector.tensor_add(eacc[:], pair0[:], epair[:])
                        else:
                            nc.vector.tensor_add(eacc[:], eacc[:], epair[:])
                        pend.append((kt - 1, epair[:, 0, :], ctx_ps, blk_i))
                        pend.append((kt, epair[:, 1, :], ctx_ps, blk_i))
                    while len(pend) > STAGGER:
                        flush_av()
                    # tb3 q-tails: ssqs in block 1, final chains block 2
                    if blk_i == 1 and kt in (4, 8):
                        flush_pe(1)
                    if blk_i == 2 and kt == 6:
                        flush_pe(1)
                    # spread Wo: one output tile (4 matmuls) per slot
                    if qb > 0 and kt % 2 == 1:
                        if (h == 0 and kt >= 9) or h == 1 or \
                           (h == 2 and kt <= 7):
                            flush_wo(1)

                def norm_job(h=h, qb=qb, qsl=qsl, ctx_ps=ctx_ps, eacc=eacc):
                    sum_ps = psum.tile([1, TBS], F32, tag="pG", bufs=1,
                                       name=f"sum_{h}_{qb}")
                    nc.tensor.matmul(sum_ps[:], onesb_col[:], eacc[:, 0, :],
                                     start=True, stop=False)
                    nc.tensor.matmul(sum_ps[:], onesb_col[:], eacc[:, 1, :],
                                     start=False, stop=True)
                    recip = rows.tile([1, TBS], F32, tag="recip",
                                      name=f"recip_{h}_{qb}")
                    nc.vector.reciprocal_approx_fast(out=recip[:],
                                                     in_=sum_ps[:])
                    recipb = scratch.tile([128, TBS], F32, tag="bcast",
                                          bufs=6, name=f"recipb_{h}_{qb}")
                    nc.gpsimd.partition_broadcast(recipb[:], recip[:])
                    nc.vector.tensor_mul(ctxT[h][:, qsl], ctx_ps, recipb[:])
                norm_jobs[blk_i] = norm_job
            pending_wo.extend(emit_wo(qb))

        while pend:
            flush_av()
        for i in sorted(list(norm_jobs)):
            norm_jobs.pop(i)()
        flush_wo(len(pending_wo), drain=True)

    nc.compile()
    return nc


def _prep_inputs(hidden_states, positions, Wq, Wk, Wv, Wo, q_norm_w, k_norm_w):
    hidden_states = np.asarray(hidden_states, dtype=np.float32)
    positions = np.asarray(positions)
    Wq = np.asarray(Wq, dtype=np.float32)
    Wk = np.asarray(Wk, dtype=np.float32)
    Wv = np.asarray(Wv, dtype=np.float32)
    Wo = np.asarray(Wo, dtype=np.float32)
    q_norm_w = np.asarray(q_norm_w, dtype=np.float32)
    k_norm_w = np.asarray(k_norm_w, dtype=np.float32)

    import ml_dtypes

    # head-dim permutation: pair (j, j+64) -> partitions (2j, 2j+1)
    perm = np.empty(D, dtype=np.int64)
    perm[0::2] = np.arange(HALF)
    perm[1::2] = np.arange(HALF) + HALF

    # permute projection output columns per head
    Wq_p = Wq.reshape(HID, NH, D)[:, :, perm].reshape(HID, NH * D)
    Wk_p = Wk.reshape(HID, NKV, D)[:, :, perm].reshape(HID, NKV * D)
    qnw_p = q_norm_w[perm]
    knw_p = k_norm_w[perm]

    inv_freq = THETA ** (-np.arange(HALF, dtype=np.float32) / HALF)
    in_maps = []
    for c in range(DP * TP):
        b, g = divmod(c, TP)
        freqs = positions[b].astype(np.float32)[:, None] * inv_freq[None, :]
        cos = np.cos(freqs).T.astype(np.float32)      # [64, S]
        sin = np.sin(freqs).T.astype(np.float32)
        # per-pair layout: row 2j/2j+1 both carry cos_j; sin row 2j is
        # -sin_j (even gets -x_odd*sin) and row 2j+1 is +sin_j
        cos2 = np.empty((D, S), dtype=np.float32)
        sin2 = np.empty((D, S), dtype=np.float32)
        cos2[0::2] = cos
        cos2[1::2] = cos
        sin2[0::2] = -sin
        sin2[1::2] = sin
        in_maps.append({
            "xt": np.ascontiguousarray(hidden_states[b].T).astype(PROJ_NP),
            "wq": np.ascontiguousarray(Wq_p[:, g * DQ:(g + 1) * DQ]).astype(PROJ_NP),
            "wk": np.ascontiguousarray(Wk_p[:, g * D:(g + 1) * D]).astype(PROJ_NP),
            "wv": np.ascontiguousarray(Wv[:, g * D:(g + 1) * D]).astype(PROJ_NP),
            "wo": np.ascontiguousarray(Wo[g * DQ:(g + 1) * DQ, :]).astype(MM_NP),
            "cos2": np.ascontiguousarray(cos2).astype(ml_dtypes.bfloat16),
            "sin2": np.ascontiguousarray(sin2).astype(ml_dtypes.bfloat16),
            "qnw": np.ascontiguousarray(qnw_p[:, None]),
            "knw": np.ascontiguousarray(knw_p[:, None]),
            "onesb": np.ones((128, 1), dtype=ml_dtypes.bfloat16),
        })
    return in_maps


def _run(inputs, trace=False):
    skip_w = bool(np.allclose(inputs["q_norm_w"], 1.0)
                  and np.allclose(inputs["k_norm_w"], 1.0))
    key = ("nc", skip_w)
    if key not in _cache:
        _cache[key] = _build(skip_w)
    nc = _cache[key]
    in_maps = _prep_inputs(**inputs)
    res = run_bass_kernel_spmd(nc, in_maps, core_ids=list(range(DP * TP)),
                               trace=trace)
    out = np.zeros((B, S, HID), dtype=np.float32)
    for c in range(DP * TP):
        out[c // TP] += res.results[c]["out"]
    return out, res


def kernel(**inputs):
    out, _ = _run(inputs, trace=False)
    return out


# revision 13
# speedup vs baseline: 1.2075x; 1.0164x over previous
"""Trainium2 Bass kernel for nn_DFlashAttention_43774306681111.

Full-attention transformer block: QKV projection + per-head RMSNorm + neox
RoPE + GQA softmax attention (non-causal) + output projection.

Sharding (8 cores): 2-way data parallel over batch x 4-way tensor parallel
over heads. Core c handles batch c//4 and head group g=c%4 (q heads
4g..4g+3, kv head g). Each core computes a partial output [S, HID]
(its heads' contribution through Wo); the host sums the 4 partials per
batch. No device collectives.

Device layout: activations kept transposed ([dim, token], dim on
partitions) so every matmul contracts on the partition axis:
  Q^T = Wq_tile^T @ X^T          (stationary Wq tile, moving X^T tile)
  S^T[k,q] = K^T_tile^T @ Q^T    (contraction d=128, one matmul per tile)
  softmax over k (= partitions): one exp per KEY-TILE PAIR ([128,1024]
    ACT instruction over a 2-bank psum tile, amortizing the fixed
    activation overhead); the denominator is accumulated with bf16 DVE
    adds over exp pairs and reduced with two ones-vector matmuls per
    block (instead of one per key tile).
  ctx^T[d,q] = V_tile^T @ expS^T (V stationary [k_tok, d])
  out[tok,hid] = ctxT_tile^T @ Wo
V is transposed [d,tok]->[tok,d] by the DMA xbar (dma_start_transpose,
one per token block) -- no PE transposes, no psum evacuations for it.

RoPE: the head dims are PERMUTED host-side (Wq/Wk columns, cos/sin
tables, norm weights) so the rotation pair (i, i+64) sits on adjacent
partitions (2i, 2i+1). The half-swap is then a single DVE stream_shuffle
(even<->odd within each 32-partition quadrant) instead of two SBUF->SBUF
DMAs. Scores/outputs are unchanged because QK contracts over the (same)
permutation of both q and k, and v/ctx/Wo are untouched.

PSUM budget (8 banks): three 2-bank pair tiles (pA: phase-A acc q0/q1
then QK even pairs; pC: acc q2/q3 then QK odd pairs; pE: acc k/v then
the double-buffered ctx accumulator) + pG (softmax sums) + pH (rmsnorm
ssq + Wo output tiles).

Engine discipline: ACT runs ONLY exp in the attention phase (plus the
batched tb3 q-tail sqrts at one point and the Wo psum copies, which
share Exp's table set) so its table never thrashes. All other PSUM
evacuations go to DVE. Wo matmuls are spread one output tile (4
matmuls) per slot through the next q-block's PE stream so exp never
starves behind a long Wo burst.
"""
import numpy as np
from contextlib import ExitStack

import concourse.bass as bass
import concourse.tile as tile
from concourse import bacc, mybir
from concourse.bass_utils import run_bass_kernel_spmd

B, S, HID = 2, 2048, 2048
NH, NKV, D = 16, 4, 128
EPS = 1e-6
THETA = 1000000.0
SCALE = D ** -0.5

TP = 4                 # tensor-parallel groups (heads)
DP = 2                 # data-parallel over batch
HG = NH // TP          # q heads per core = 4
DQ = HG * D            # 512 q-proj cols per core
HALF = D // 2          # 64

F32 = mybir.dt.float32
F32R = mybir.dt.float32r
BF16 = mybir.dt.bfloat16
FP16 = mybir.dt.float16

MM_DT = F32R           # ctx / Wo matmul operand dtype
MM_NP = np.float32
PROJ_DT = FP16         # projection operand dtype (halves phase A DMA)
PROJ_NP = np.float16

HT = HID // 128        # 16 hid tiles
TBS = 512              # token block size
NTB = S // TBS         # 4 token blocks
KT = S // 128          # 16 key tiles
QB = S // TBS          # 4 query blocks
NDT = HG + 2           # 6 projection outputs: q0..q3, k, v^T

STAGGER = 5            # AV matmul lag behind QK/exp

SWAP_MASK = [i ^ 1 for i in range(32)]   # even<->odd partition swap

_cache = {}


def _build(skip_w=False):
    nc = bacc.Bacc(None, target_bir_lowering=False, debug=False)

    xt = nc.dram_tensor("xt", [HID, S], PROJ_DT, kind="ExternalInput")
    wq = nc.dram_tensor("wq", [HID, DQ], PROJ_DT, kind="ExternalInput")
    wk = nc.dram_tensor("wk", [HID, D], PROJ_DT, kind="ExternalInput")
    wv = nc.dram_tensor("wv", [HID, D], PROJ_DT, kind="ExternalInput")
    wo = nc.dram_tensor("wo", [DQ, HID], MM_DT, kind="ExternalInput")
    cos2 = nc.dram_tensor("cos2", [D, S], BF16, kind="ExternalInput")
    sin2 = nc.dram_tensor("sin2", [D, S], BF16, kind="ExternalInput")
    qnw = nc.dram_tensor("qnw", [D, 1], F32, kind="ExternalInput")
    knw = nc.dram_tensor("knw", [D, 1], F32, kind="ExternalInput")
    onesb_d = nc.dram_tensor("onesb", [128, 1], BF16, kind="ExternalInput")
    out = nc.dram_tensor("out", [S, HID], F32, kind="ExternalOutput")

    with tile.TileContext(nc) as tc, ExitStack() as ctx:
        const = ctx.enter_context(tc.tile_pool(name="const", bufs=1))
        big = ctx.enter_context(tc.tile_pool(name="big", bufs=1))
        blk = ctx.enter_context(tc.tile_pool(name="blk", bufs=8))
        outp = ctx.enter_context(tc.tile_pool(name="outp", bufs=3))
        scratch = ctx.enter_context(tc.tile_pool(name="scratch", bufs=2))
        rows = ctx.enter_context(tc.tile_pool(name="rows", bufs=2))
        psum = ctx.enter_context(tc.tile_pool(name="psum", bufs=1, space="PSUM"))

        # ---- constants ----
        onesb_col = const.tile([128, 1], BF16)
        nc.scalar.dma_start(out=onesb_col[:], in_=onesb_d[:])
        eps1 = const.tile([1, 1], F32)
        nc.vector.memset(eps1, EPS)
        qnw_sb = const.tile([D, 1], F32)
        nc.scalar.dma_start(out=qnw_sb[:], in_=qnw[:])
        knw_sb = const.tile([D, 1], F32)
        nc.scalar.dma_start(out=knw_sb[:], in_=knw[:])

        # ---- resident weights / big activations (tag-shared slots) ----
        wq_sb = big.tile([128, HT, DQ], PROJ_DT, tag="bigw")
        wk_sb = big.tile([128, HT, D], PROJ_DT, tag="wk")
        wv_sb = big.tile([128, HT, D], PROJ_DT, tag="wv")
        cos_sb = big.tile([D, S], BF16, tag="cosb")
        sin_sb = big.tile([D, S], BF16, tag="sinb")

        qT = big.tile([D, HG, S], BF16, tag="qT")        # Q^T per head
        kT = big.tile([D, S], BF16, tag="kT")            # K^T
        vT = big.tile([D, S], BF16, tag="vT")            # V^T (pre-transpose)
        v_sb = big.tile([128, KT, D], BF16, tag="v")     # V [tok, d] tiles

        def stationary(ht, dt):
            if dt < HG:
                return wq_sb[:, ht, dt * D:(dt + 1) * D]
            if dt == HG:
                return wk_sb[:, ht, :]
            return wv_sb[:, ht, :]

        # Deferred PE work from the rmsnorm tails: one ssq matmul + sqrt +
        # recip + broadcast + rope chain per projection output. Flushed at
        # spread-out points of the later PE stream.
        pending_pe = []

        def flush_pe(k=1):
            for _ in range(min(k, len(pending_pe))):
                pending_pe.pop(0)()

        # ---- phase A: projections ----
        for tb in range(NTB):
            tsl = slice(tb * TBS, (tb + 1) * TBS)
            pairs = [psum.tile([128, 2, TBS], F32, tag=t,
                               name=f"accp_{tb}_{t}")
                     for t in ("pA", "pC", "pE")]
            accs = [pairs[dt // 2][:, dt % 2, :] for dt in range(NDT)]
            for ht in range(HT):
                if tb == 0:
                    hsl = slice(ht * 128, (ht + 1) * 128)
                    nc.scalar.dma_start(out=wq_sb[:, ht, :], in_=wq[hsl, :])
                    nc.scalar.dma_start(out=wk_sb[:, ht, :], in_=wk[hsl, :])
                    nc.scalar.dma_start(out=wv_sb[:, ht, :], in_=wv[hsl, :])
                if ht == 1:
                    nc.scalar.dma_start(out=cos_sb[:, tsl], in_=cos2[:, tsl])
                    nc.scalar.dma_start(out=sin_sb[:, tsl], in_=sin2[:, tsl])
                if ht in (4, 6, 8, 10, 12):
                    flush_pe(1)  # previous tb's rmsnorm/rope tails
                xt_t = blk.tile([128, TBS], PROJ_DT, tag="xt", bufs=12,
                                name=f"xt_{tb}_{ht}")
                nc.sync.dma_start(out=xt_t[:], in_=xt[ht * 128:(ht + 1) * 128, tsl])
                for dt in range(NDT):
                    nc.tensor.matmul(accs[dt], stationary(ht, dt), xt_t[:],
                                     start=(ht == 0), stop=(ht == HT - 1))

            # ---- end of tb: evacuate psums fast (k first: its tail ssq
            # flushes earliest), then queue tail work ----
            raws = {}
            q2s = {}
            for i, dt in enumerate([HG, 0, 1, NDT - 1, 2, 3]):
                acc = accs[dt]
                if dt == NDT - 1:
                    # v^T: evac, then DMA-transpose this tb's chunk into
                    # v_sb [tok, d] on an idle DMA queue (no PE transposes)
                    nc.vector.tensor_copy(vT[:, tsl], acc)
                    nc.sync.dma_start_transpose(
                        v_sb[:, tb * (TBS // 128):(tb + 1) * (TBS // 128), :],
                        vT[:, tsl])
                    continue
                raw = scratch.tile([128, TBS], BF16, tag="raw", bufs=10,
                                   name=f"raw_{tb}_{dt}")
                nc.vector.tensor_copy(raw[:], acc)
                q2 = scratch.tile([128, TBS], BF16, tag="q2", bufs=7,
                                  name=f"q2_{tb}_{dt}")
                nc.gpsimd.tensor_mul(q2[:], raw[:], raw[:])
                raws[dt] = raw
                q2s[dt] = q2

            # Tail work, staged to avoid cross-engine ping-pong: the five
            # ssq matmuls (psum pH) + sqrts land in a shared [1,5,TBS] row
            # tile; reciprocals are batched (k+q0, then q1..q3); the rope
            # chains run DVE-only with one gps broadcast per tail.
            tail_dts = [HG, 0, 1, 2, 3]
            rst5 = rows.tile([1, 5, TBS], F32, tag="rst", bufs=1,
                             name=f"rst5_{tb}")
            rstr5 = rows.tile([1, 5, TBS], F32, tag="rstr", bufs=1,
                              name=f"rstr5_{tb}")

            def emit_ssq(i, dt, tb=tb, rst5=rst5, q2s=q2s):
                ssq = psum.tile([1, TBS], F32, tag="pH",
                                bufs=1, name=f"ssq_{tb}_{dt}")
                nc.tensor.matmul(ssq[:], onesb_col[:], q2s[dt][:],
                                 start=True, stop=True)
                nc.scalar.activation(rst5[0:1, i, :], ssq[:],
                                     mybir.ActivationFunctionType.Sqrt,
                                     scale=1.0 / D, bias=eps1[:])

            def emit_chain(i, dt, tb=tb, tsl=tsl, raws=raws, rstr5=rstr5):
                raw = raws[dt]
                rstdb = scratch.tile([128, TBS], F32, tag="bcast",
                                     bufs=6, name=f"rstdb_{tb}_{dt}")
                nc.gpsimd.partition_broadcast(rstdb[:], rstr5[0:1, i, :])
                xsw = scratch.tile([128, TBS], BF16, tag="xsw", bufs=4,
                                   name=f"xsw_{tb}_{dt}")
                nc.vector.stream_shuffle(xsw[:], raw[:], SWAP_MASK)
                tmp = scratch.tile([128, TBS], BF16, tag="tmp", bufs=4,
                                   name=f"tmp_{tb}_{dt}")
                nc.vector.tensor_mul(tmp[:], raw[:], cos_sb[:, tsl])
                sv = scratch.tile([128, TBS], BF16, tag="sv", bufs=4,
                                  name=f"sv_{tb}_{dt}")
                nc.vector.tensor_mul(sv[:], xsw[:], sin_sb[:, tsl])
                qro = scratch.tile([128, TBS], BF16, tag="qro", bufs=4,
                                   name=f"qro_{tb}_{dt}")
                nc.vector.tensor_add(qro[:], tmp[:], sv[:])
                dest = qT[:, dt, tsl] if dt < HG else kT[:, tsl]
                nc.vector.tensor_mul(dest, qro[:], rstdb[:])
                if not skip_w:
                    w_ap = qnw_sb if dt < HG else knw_sb
                    nc.vector.tensor_scalar_mul(dest, dest, w_ap[:])

            def entry0(tb=tb):
                emit_ssq(0, HG)
            def entry1(tb=tb, rst5=rst5, rstr5=rstr5):
                emit_ssq(1, 0)
                nc.vector.reciprocal_approx_fast(out=rstr5[0:1, 0:2, :],
                                                 in_=rst5[0:1, 0:2, :])
                emit_chain(0, HG)
                emit_chain(1, 0)
            def entry2(tb=tb):
                emit_ssq(2, 1)
            def entry3(tb=tb):
                emit_ssq(3, 2)
            def entry4(tb=tb, rst5=rst5, rstr5=rstr5):
                emit_ssq(4, 3)
                nc.vector.reciprocal_approx_fast(out=rstr5[0:1, 2:5, :],
                                                 in_=rst5[0:1, 2:5, :])
                emit_chain(2, 1)
                emit_chain(3, 2)
                emit_chain(4, 3)

            pending_pe.extend([entry0, entry1, entry2, entry3, entry4])

        flush_pe(5)   # all tb3 tails: sqrts before the first exp table load

        # ctx^T per head
        ctxT = [big.tile([D, S], MM_DT, tag=f"ctx{h}", name=f"ctxT_{h}")
                for h in range(HG)]

        # wo loads overlap the first attention blocks ("bigw" frees after
        # the last projection matmul)
        wo_sb = big.tile([128, HG, HID], MM_DT, tag="bigw")
        for ct in range(HG):
            nc.scalar.dma_start(out=wo_sb[:, ct, :],
                                in_=wo[ct * 128:(ct + 1) * 128, :])

        # ---- phase B: attention (qb-major) with Wo folded in ----
        # double-buffered ctx accumulator: halves of one 2-bank pair tile
        ctx2 = psum.tile([128, 2, TBS], F32, tag="pE", name="ctx2")

        pending_wo = []

        def emit_wo(qb):
            thunks = []
            for tt in range(qb * (TBS // 128), (qb + 1) * (TBS // 128)):
                for hc in range(HID // TBS):
                    def thunk(tag, evac, tt=tt, hc=hc):
                        o_ps = psum.tile([128, TBS], F32, tag=tag,
                                         name=f"o_{tt}_{hc}")
                        for ct in range(HG):
                            nc.tensor.matmul(
                                o_ps[:],
                                ctxT[ct][:, tt * 128:(tt + 1) * 128],
                                wo_sb[:, ct, hc * TBS:(hc + 1) * TBS],
                                start=(ct == 0), stop=(ct == HG - 1))
                        o_sb = outp.tile([128, TBS], F32, tag="osb",
                                         name=f"osb_{tt}_{hc}")
                        if evac == 0:
                            nc.scalar.copy(o_sb[:], o_ps[:])
                        else:
                            nc.vector.tensor_copy(o_sb[:], o_ps[:])
                        nc.sync.dma_start(
                            out=out[tt * 128:(tt + 1) * 128,
                                    hc * TBS:(hc + 1) * TBS],
                            in_=o_sb[:])
                    thunks.append(thunk)
            return thunks

        wo_flip = [0]

        def flush_wo(k, drain=False):
            for _ in range(min(k, len(pending_wo))):
                j = wo_flip[0] = wo_flip[0] + 1
                pending_wo.pop(0)(["pH", "pG"][j % 2], j % 2)

        # cross-block AV stagger; entries: (kt, e_ap, ctx_ap, blk_i)
        pend = []
        norm_jobs = {}

        def flush_av():
            kt0, e0, c_ps, bi = pend.pop(0)
            nc.tensor.matmul(c_ps, v_sb[:, kt0, :], e0,
                             start=(kt0 == 0), stop=(kt0 == KT - 1))
            if kt0 == KT - 1 and bi in norm_jobs:
                norm_jobs.pop(bi)()

        for qb in range(QB):
            qsl = slice(qb * TBS, (qb + 1) * TBS)
            for h in range(HG):
                blk_i = qb * HG + h
                ctx_ps = ctx2[:, blk_i % 2, :]
                eacc = scratch.tile([128, 2, TBS], BF16, tag="eacc", bufs=2,
                                    name=f"eacc_{h}_{qb}")
                pair0 = None
                epair = None
                s_pair = None

                for kt in range(KT):
                    if kt % 2 == 0:
                        s_pair = psum.tile([128, 2, TBS], F32,
                                           tag=["pA", "pC"][(kt // 2) % 2],
                                           name=f"s_{h}_{qb}_{kt}")
                        epair = blk.tile([128, 2, TBS], BF16, tag="blk",
                                         bufs=4, name=f"e_{h}_{qb}_{kt}")
                    nc.tensor.matmul(s_pair[:, kt % 2, :],
                                     kT[:, kt * 128:(kt + 1) * 128],
                                     qT[:, h, qsl], start=True, stop=True)
                    if kt % 2 == 1:
                        # one exp + one accumulator add per key-tile pair
                        nc.scalar.activation(epair[:], s_pair[:],
                                             mybir.ActivationFunctionType.Exp,
                                             scale=SCALE)
                        if kt == 1:
                            pair0 = epair
                        elif kt == 3:
                            nc.vector.tensor_add(eacc[:], pair0[:], epair[:])
                        else:
                            nc.vector.tensor_add(eacc[:], eacc[:], epair[:])
                        pend.append((kt - 1, epair[:, 0, :], ctx_ps, blk_i))
                        pend.append((kt, epair[:, 1, :], ctx_ps, blk_i))
                    while len(pend) > STAGGER:
                        flush_av()
                    # spread Wo: one output tile (4 matmuls) per slot
                    if qb > 0 and kt % 2 == 1:
                        if (h == 0 and kt >= 9) or h == 1 or \
                           (h == 2 and kt <= 7):
                            flush_wo(1)

                def norm_job(h=h, qb=qb, qsl=qsl, ctx_ps=ctx_ps, eacc=eacc):
                    sum_ps = psum.tile([1, TBS], F32, tag="pG", bufs=1,
                                       name=f"sum_{h}_{qb}")
                    nc.tensor.matmul(sum_ps[:], onesb_col[:], eacc[:, 0, :],
                                     start=True, stop=False)
                    nc.tensor.matmul(sum_ps[:], onesb_col[:], eacc[:, 1, :],
                                     start=False, stop=True)
                    recip = rows.tile([1, TBS], F32, tag="recip",
                                      name=f"recip_{h}_{qb}")
                    nc.vector.reciprocal_approx_fast(out=recip[:],
                                                     in_=sum_ps[:])
                    recipb = scratch.tile([128, TBS], F32, tag="bcast",
                                          bufs=6, name=f"recipb_{h}_{qb}")
                    nc.gpsimd.partition_broadcast(recipb[:], recip[:])
                    nc.vector.tensor_mul(ctxT[h][:, qsl], ctx_ps, recipb[:])
                norm_jobs[blk_i] = norm_job
            pending_wo.extend(emit_wo(qb))

        while pend:
            flush_av()
        for i in sorted(list(norm_jobs)):
            norm_jobs.pop(i)()
        flush_wo(len(pending_wo), drain=True)

    nc.compile()
    return nc


def _prep_inputs(hidden_states, positions, Wq, Wk, Wv, Wo, q_norm_w, k_norm_w):
    hidden_states = np.asarray(hidden_states, dtype=np.float32)
    positions = np.asarray(positions)
    Wq = np.asarray(Wq, dtype=np.float32)
    Wk = np.asarray(Wk, dtype=np.float32)
    Wv = np.asarray(Wv, dtype=np.float32)
    Wo = np.asarray(Wo, dtype=np.float32)
    q_norm_w = np.asarray(q_norm_w, dtype=np.float32)
    k_norm_w = np.asarray(k_norm_w, dtype=np.float32)

    import ml_dtypes

    # head-dim permutation: pair (j, j+64) -> partitions (2j, 2j+1)
    perm = np.empty(D, dtype=np.int64)
    perm[0::2] = np.arange(HALF)
    perm[1::2] = np.arange(HALF) + HALF

    # permute projection output columns per head
    Wq_p = Wq.reshape(HID, NH, D)[:, :, perm].reshape(HID, NH * D)
    Wk_p = Wk.reshape(HID, NKV, D)[:, :, perm].reshape(HID, NKV * D)
    qnw_p = q_norm_w[perm]
    knw_p = k_norm_w[perm]

    inv_freq = THETA ** (-np.arange(HALF, dtype=np.float32) / HALF)
    in_maps = []
    for c in range(DP * TP):
        b, g = divmod(c, TP)
        freqs = positions[b].astype(np.float32)[:, None] * inv_freq[None, :]
        cos = np.cos(freqs).T.astype(np.float32)      # [64, S]
        sin = np.sin(freqs).T.astype(np.float32)
        # per-pair layout: row 2j/2j+1 both carry cos_j; sin row 2j is
        # -sin_j (even gets -x_odd*sin) and row 2j+1 is +sin_j
        cos2 = np.empty((D, S), dtype=np.float32)
        sin2 = np.empty((D, S), dtype=np.float32)
        cos2[0::2] = cos
        cos2[1::2] = cos
        sin2[0::2] = -sin
        sin2[1::2] = sin
        in_maps.append({
            "xt": np.ascontiguousarray(hidden_states[b].T).astype(PROJ_NP),
            "wq": np.ascontiguousarray(Wq_p[:, g * DQ:(g + 1) * DQ]).astype(PROJ_NP),
            "wk": np.ascontiguousarray(Wk_p[:, g * D:(g + 1) * D]).astype(PROJ_NP),
            "wv": np.ascontiguousarray(Wv[:, g * D:(g + 1) * D]).astype(PROJ_NP),
            "wo": np.ascontiguousarray(Wo[g * DQ:(g + 1) * DQ, :]).astype(MM_NP),
            "cos2": np.ascontiguousarray(cos2).astype(ml_dtypes.bfloat16),
            "sin2": np.ascontiguousarray(sin2).astype(ml_dtypes.bfloat16),
            "qnw": np.ascontiguousarray(qnw_p[:, None]),
            "knw": np.ascontiguousarray(knw_p[:, None]),
            "onesb": np.ones((128, 1), dtype=ml_dtypes.bfloat16),
        })
    return in_maps


def _run(inputs, trace=False):
    skip_w = bool(np.allclose(inputs["q_norm_w"], 1.0)
                  and np.allclose(inputs["k_norm_w"], 1.0))
    key = ("nc", skip_w)
    if key not in _cache:
        _cache[key] = _build(skip_w)
    nc = _cache[key]
    in_maps = _prep_inputs(**inputs)
    res = run_bass_kernel_spmd(nc, in_maps, core_ids=list(range(DP * TP)),
                               trace=trace)
    out = np.zeros((B, S, HID), dtype=np.float32)
    for c in range(DP * TP):
        out[c // TP] += res.results[c]["out"]
    return out, res


def kernel(**inputs):
    out, _ = _run(inputs, trace=False)
    return out
